# revision 33
# baseline (speedup 1.0000x reference)
"""GRUAggregation1d Trainium2 kernel.

Computes, for xs [B=16, 512, L=8192], z_prev [B, 128, L] (all fp32):
    q  = sigmoid(Wq@xs + Uq@z + bq)        (per position l, batch b)
    r  = sigmoid(Wr@xs + Ur@z + br)
    zt = tanh(Wz@xs + Uz@(r*z) + bz)
    out = q*z + (1-q)*zt

Sharding: data-parallel over batch. 8 cores x 2 batches each; weights
replicated.

Design (baseline ~160us -> ~123us):
- q/zt matmuls bf16 (fp8 measured out of tolerance on those paths); the
  r gate runs fp8 DoubleRow (K=256 per pass) with x64 pre-scaled weights,
  compensated via the sigmoid's scale; Ur is pre-scaled x64 in bf16 so
  its products land on the same PSUM scale. PSUM fp32, N=512 per matmul.
- Work unit: 1024-position supertile == DMA chunk (4KB xs rows, 2KB fp8
  rows). 5-deep input prefetch so the DMA stream never starves the PE
  after the initial fill. xs/z triggers on the sync queue, x8/out on
  gpsimd, scalar reserved for ACT (DMA triggers cost ~0.7us each).
- Per supertile the r gate is computed FIRST so r*z (DVE, bf16) is ready
  ~1.3us before the Uz matmuls at the stream tail -- no cross-supertile
  software pipeline needed, and the PE never waits on the r->r*z chain.
- Consecutive matmuls always target different PSUM banks; the two
  half-tile matmuls per weight are adjacent (stationary-weight reuse;
  LDWEIGHTS overlaps matmuls in HW).
- ACT ops are 1024 wide with fused bias (PSUM fp32 in, bf16 out); the
  combine runs in bf16 on DVE (2x rate); z_prev and out are bf16 in HBM
  (host casts), halving that traffic.
- All bf16 weights in one [128,1920] DMA; fp8 r-weights + biases in two
  tiny DMAs.
"""

from contextlib import ExitStack

import ml_dtypes
import numpy as np

import concourse.bass as bass
import concourse.mybir as mybir
import concourse.tile as tile
from concourse import bacc
from concourse.bass_utils import run_bass_kernel_spmd

B, IN_DIM, WIDTH, L = 16, 512, 128, 8192
N_CORES = 8
B_PER = B // N_CORES          # batches per core
KC = IN_DIM // 128            # K chunks for the W matmuls
NT = 512                      # positions per matmul (one PSUM bank)
ST = 1024                     # supertile / DMA chunk positions
F32 = mybir.dt.float32
BF16 = mybir.dt.bfloat16
FP8 = mybir.dt.float8e4
WRS = 64.0                    # r-gate fp8 weight pre-scale

_module_cache = {}


def _build():
    key = ("v11", ST)
    if key in _module_cache:
        return _module_cache[key]

    nc = bacc.Bacc("TRN2", target_bir_lowering=False, debug=False,
                   num_devices=N_CORES)

    xs_d = nc.dram_tensor("xs", [B_PER, IN_DIM, L], BF16, kind="ExternalInput").ap()
    zp_d = nc.dram_tensor("zp", [B_PER, WIDTH, L], BF16, kind="ExternalInput").ap()
    wp_d = nc.dram_tensor("wp", [128, 1920], BF16, kind="ExternalInput").ap()
    w8_d = nc.dram_tensor("w8", [128, 2, 2, 128], FP8, kind="ExternalInput").ap()
    bp_d = nc.dram_tensor("bp", [128, 3], F32, kind="ExternalInput").ap()
    out_d = nc.dram_tensor("out", [B_PER, WIDTH, L], BF16,
                           kind="ExternalOutput").ap()

    # [b, (k p), l] -> [b, p, k, l]: a chunk slice is a [128, KC, ST] DMA
    # with contiguous rows
    xs_r = xs_d.rearrange("b (k p) l -> b p k l", p=128)

    with tile.TileContext(nc) as tc, ExitStack() as ctx:
        wpool = ctx.enter_context(tc.tile_pool(name="weights", bufs=1))
        io = ctx.enter_context(tc.tile_pool(name="io", bufs=5))
        obuf = ctx.enter_context(tc.tile_pool(name="obuf", bufs=2))
        acts = ctx.enter_context(tc.tile_pool(name="acts", bufs=2))
        ps_q = ctx.enter_context(tc.tile_pool(name="ps_q", bufs=2,
                                              space="PSUM"))
        ps_rz = ctx.enter_context(tc.tile_pool(name="ps_rz", bufs=1,
                                               space="PSUM"))

        # weights first (small, every matmul needs them), spread over the
        # three DMA-capable queues so the triggers don't serialize.
        w_sb = wpool.tile([128, 1920], BF16, tag="wp")
        nc.sync.dma_start(w_sb[:], wp_d[:])
        w8_sb = wpool.tile([128, 2, 2, 128], FP8, tag="w8")
        nc.gpsimd.dma_start(w8_sb[:], w8_d[:])
        b_sb = wpool.tile([128, 3], F32, tag="bp")
        nc.scalar.dma_start(b_sb[:], bp_d[:])

        # weight slices: per gate g (0=q,1=r,2=z): W chunks at
        # [:, g*512 + k*128 : +128], U at [:, 1536 + g*128 : +128]
        def wslice(g, k):
            return w_sb[:, g * 512 + k * 128: g * 512 + (k + 1) * 128]

        def uslice(g):
            return w_sb[:, 1536 + g * 128: 1536 + (g + 1) * 128]

        n_chunks = B_PER * (L // ST)
        tiles = {}

        def load_chunk(m):
            """Input DMAs for chunk m, plus the on-chip bf16->fp8 cast of
            xs on the (otherwise idle) GpSimd engine. The cast replaces an
            8.4MB HBM stream; it runs ~2 chunks ahead of use."""
            mb, ml = divmod(m, L // ST)
            ml *= ST
            xs_t = io.tile([128, KC, ST], BF16, tag="xs_t")
            nc.sync.dma_start(xs_t[:], xs_r[mb][:, :, ml:ml + ST])
            z_t = io.tile([128, ST], BF16, tag="z_t")
            nc.sync.dma_start(z_t[:], zp_d[mb][:, ml:ml + ST])
            x8_t = None
            if m > 0:  # chunk 0's r gate runs on the bf16 weights
                x8_t = io.tile([128, KC, ST], FP8, tag="x8_t")
                nc.gpsimd.tensor_scalar_add(x8_t[:], xs_t[:], 0.0)
            tiles[m] = (xs_t, x8_t, z_t)

        LOOKAHEAD = 2
        for m in range(min(LOOKAHEAD + 1, n_chunks)):
            load_chunk(m)

        for n in range(n_chunks):
            if n + LOOKAHEAD + 1 < n_chunks:
                load_chunk(n + LOOKAHEAD + 1)
            b_i, l0 = divmod(n, L // ST)
            l0 *= ST
            xs_t, x8_t, z_t = tiles.pop(n)
            ob = obuf.tile([128, ST], BF16, tag="ob")

            q_ps = ps_q.tile([128, ST], F32, tag="q_ps")
            r_ps = ps_rz.tile([128, ST], F32, tag="r_ps")
            zt_ps = ps_rz.tile([128, ST], F32, tag="zt_ps")

            def r_gate():
                # r gate: its sigmoid + r*z run on ACT/DVE while the q/zt
                # matmuls stream, so rz is ready well before the Uz matmuls
                # at the end of this supertile's stream. Chunk 0 uses the
                # bf16 Wr (pre-scaled x64 like the fp8 path) so the very
                # first matmul is gated by the xs DMA, not the later x8.
                if n == 0:
                    for k in range(KC):
                        w = wslice(1, k)
                        for h in range(2):
                            nc.tensor.matmul(
                                r_ps[:, h * NT:(h + 1) * NT], w,
                                xs_t[:, k, h * NT:(h + 1) * NT],
                                start=(k == 0), stop=False)
                else:
                    for k2 in range(2):
                        for h in range(2):
                            nc.tensor.matmul(
                                r_ps[:, h * NT:(h + 1) * NT], w8_sb[:, k2],
                                x8_t[:, 2 * k2:2 * k2 + 2,
                                     h * NT:(h + 1) * NT],
                                start=(k2 == 0), stop=False,
                                perf_mode=mybir.MatmulPerfMode.DoubleRow)
                ur = uslice(1)
                for h in range(2):
                    nc.tensor.matmul(r_ps[:, h * NT:(h + 1) * NT], ur,
                                     z_t[:, h * NT:(h + 1) * NT],
                                     start=False, stop=True)

            def q_gate():
                for k in range(KC):
                    w = wslice(0, k)
                    for h in range(2):
                        nc.tensor.matmul(
                            q_ps[:, h * NT:(h + 1) * NT], w,
                            xs_t[:, k, h * NT:(h + 1) * NT],
                            start=(k == 0), stop=False)
                uq = uslice(0)
                for h in range(2):
                    nc.tensor.matmul(q_ps[:, h * NT:(h + 1) * NT], uq,
                                     z_t[:, h * NT:(h + 1) * NT],
                                     start=False, stop=True)

            r_gate()
            q_gate()
            r_s = acts.tile([128, ST], BF16, tag="r_s")
            nc.scalar.activation(r_s[:], r_ps[:],
                                 mybir.ActivationFunctionType.Sigmoid,
                                 bias=b_sb[:, 1:2], scale=1.0 / WRS)
            rz = acts.tile([128, ST], BF16, tag="rz")
            nc.vector.tensor_mul(rz[:], r_s[:], z_t[:])
            q_s = acts.tile([128, ST], BF16, tag="q_s")
            nc.scalar.activation(q_s[:], q_ps[:],
                                 mybir.ActivationFunctionType.Sigmoid,
                                 bias=b_sb[:, 0:1])

            # ---- zt gate: W part, then Uz@(r*z) at the stream tail
            for k in range(KC):
                w = wslice(2, k)
                for h in range(2):
                    nc.tensor.matmul(
                        zt_ps[:, h * NT:(h + 1) * NT], w,
                        xs_t[:, k, h * NT:(h + 1) * NT],
                        start=(k == 0), stop=False)
            uz = uslice(2)
            for h in range(2):
                nc.tensor.matmul(zt_ps[:, h * NT:(h + 1) * NT], uz,
                                 rz[:, h * NT:(h + 1) * NT],
                                 start=False, stop=True)

            # ---- epilogue: out = zt + q*(z - zt), bf16 on DVE. The last
            # chunk runs it in 512-halves so the serial tail chain
            # (tanh -> sub -> mul -> add -> store) pipelines.
            halves = ((0, ST),) if n < n_chunks - 1 else ((0, NT), (NT, NT))
            for h0, hw_ in halves:
                zt_s = acts.tile([128, hw_], BF16, tag=f"zt_s{h0}")
                nc.scalar.activation(zt_s[:], zt_ps[:, h0:h0 + hw_],
                                     mybir.ActivationFunctionType.Tanh,
                                     bias=b_sb[:, 2:3])
                diff = acts.tile([128, hw_], BF16, tag=f"diff{h0}")
                nc.vector.tensor_sub(diff[:], z_t[:, h0:h0 + hw_], zt_s[:])
                prod = acts.tile([128, hw_], BF16, tag=f"prod{h0}")
                nc.vector.tensor_mul(prod[:], q_s[:, h0:h0 + hw_], diff[:])
                nc.vector.tensor_add(ob[:, h0:h0 + hw_], zt_s[:], prod[:])
                nc.gpsimd.dma_start(out_d[b_i][:, l0 + h0:l0 + h0 + hw_],
                                    ob[:, h0:h0 + hw_])

    nc.compile()
    _module_cache[key] = nc
    return nc


def _pack_weights(inputs):
    # wp [128, 1920] bf16: per partition p:
    #   [g=q,r,z][k=0..3]: wp[p, g*512+k*128+o] = Wg_w[o, k*128+p]
    #   [g]: wp[p, 1536+g*128+o] = Ug_w[o, p]   (Ur pre-scaled by WRS)
    wp = np.empty((128, 1920), np.float32)
    bp = np.empty((128, 3), np.float32)
    for g, (wn, un, wbn, ubn) in enumerate((
        ("Wq_w", "Uq_w", "Wq_b", "Uq_b"),
        ("Wr_w", "Ur_w", "Wr_b", "Ur_b"),
        ("Wz_w", "Uz_w", "Wz_b", "Uz_b"),
    )):
        w = np.asarray(inputs[wn], np.float32)       # [128 out, 512 in]
        # Wr is pre-scaled like its fp8 twin (x64 is exact in bf16) --
        # chunk 0's r gate runs on this bf16 copy.
        ws = WRS if g == 1 else 1.0
        wp[:, g * 512:(g + 1) * 512] = ws * (
            w.reshape(128, KC, 128).transpose(2, 1, 0).reshape(128, 512))
        us = WRS if g == 1 else 1.0
        wp[:, 1536 + g * 128: 1536 + (g + 1) * 128] = (
            us * np.asarray(inputs[un], np.float32).T)
        bp[:, g] = (np.asarray(inputs[wbn], np.float32)
                    + np.asarray(inputs[ubn], np.float32))
    # w8 [128, k2, j, o] fp8: WRS * Wr_w[o, (2*k2+j)*128 + p]
    wr = np.asarray(inputs["Wr_w"], np.float32)      # [128, 512]
    w8 = (WRS * wr.reshape(128, 2, 2, 128).transpose(3, 1, 2, 0))
    return (np.ascontiguousarray(wp.astype(ml_dtypes.bfloat16)),
            np.ascontiguousarray(w8.astype(ml_dtypes.float8_e4m3)),
            np.ascontiguousarray(bp))


def _run(inputs, trace=False, **run_kwargs):
    xs = np.asarray(inputs["xs"], dtype=np.float32)
    zp = np.asarray(inputs["z_prev"], dtype=np.float32)
    assert xs.shape == (B, IN_DIM, L) and zp.shape == (B, WIDTH, L)
    xs_bf = np.ascontiguousarray(xs.astype(ml_dtypes.bfloat16))
    zp_bf = np.ascontiguousarray(zp.astype(ml_dtypes.bfloat16))
    wp, w8, bp = _pack_weights(inputs)

    nc = _build()
    in_maps = []
    for c in range(N_CORES):
        m = {"xs": np.ascontiguousarray(xs_bf[c * B_PER:(c + 1) * B_PER]),
             "zp": np.ascontiguousarray(zp_bf[c * B_PER:(c + 1) * B_PER]),
             "wp": wp, "w8": w8, "bp": bp}
        in_maps.append(m)

    res = run_bass_kernel_spmd(nc, in_maps, core_ids=list(range(N_CORES)),
                               trace=trace, **run_kwargs)
    out = np.concatenate(
        [np.asarray(res.results[c]["out"], dtype=np.float32)
         for c in range(N_CORES)], axis=0)
    return out, res


def kernel(**inputs):
    out, _ = _run(inputs, trace=False)
    return out


# revision 36
# speedup vs baseline: 9.6393x; 9.6393x over previous
"""GRUAggregation1d Trainium2 kernel.

Computes, for xs [B=16, 512, L=8192], z_prev [B, 128, L] (all fp32):
    q  = sigmoid(Wq@xs + Uq@z + bq)        (per position l, batch b)
    r  = sigmoid(Wr@xs + Ur@z + br)
    zt = tanh(Wz@xs + Uz@(r*z) + bz)
    out = q*z + (1-q)*zt

Sharding: data-parallel over batch. 8 cores x 2 batches each; weights
replicated.

Design (baseline ~160us -> ~123us):
- q/zt matmuls bf16 (fp8 measured out of tolerance on those paths); the
  r gate runs fp8 DoubleRow (K=256 per pass) with x64 pre-scaled weights,
  compensated via the sigmoid's scale; Ur is pre-scaled x64 in bf16 so
  its products land on the same PSUM scale. PSUM fp32, N=512 per matmul.
- Work unit: 1024-position supertile == DMA chunk (4KB xs rows, 2KB fp8
  rows). 5-deep input prefetch so the DMA stream never starves the PE
  after the initial fill. xs/z triggers on the sync queue, x8/out on
  gpsimd, scalar reserved for ACT (DMA triggers cost ~0.7us each).
- Per supertile the r gate is computed FIRST so r*z (DVE, bf16) is ready
  ~1.3us before the Uz matmuls at the stream tail -- no cross-supertile
  software pipeline needed, and the PE never waits on the r->r*z chain.
- Consecutive matmuls always target different PSUM banks; the two
  half-tile matmuls per weight are adjacent (stationary-weight reuse;
  LDWEIGHTS overlaps matmuls in HW).
- ACT ops are 1024 wide with fused bias (PSUM fp32 in, bf16 out); the
  combine runs in bf16 on DVE (2x rate); z_prev and out are bf16 in HBM
  (host casts), halving that traffic.
- All bf16 weights in one [128,1920] DMA; fp8 r-weights + biases in two
  tiny DMAs.
"""

from contextlib import ExitStack

import ml_dtypes
import numpy as np

import concourse.bass as bass
import concourse.mybir as mybir
import concourse.tile as tile
from concourse import bacc
from concourse.bass_utils import run_bass_kernel_spmd

B, IN_DIM, WIDTH, L = 16, 512, 128, 8192
N_CORES = 8
B_PER = B // N_CORES          # batches per core
KC = IN_DIM // 128            # K chunks for the W matmuls
NT = 512                      # positions per matmul (one PSUM bank)
ST = 1024                     # supertile / DMA chunk positions
F32 = mybir.dt.float32
BF16 = mybir.dt.bfloat16
FP8 = mybir.dt.float8e4
WRS = 64.0                    # r-gate fp8 weight pre-scale

_module_cache = {}


def _build():
    key = ("v11", ST)
    if key in _module_cache:
        return _module_cache[key]

    nc = bacc.Bacc("TRN2", target_bir_lowering=False, debug=False,
                   num_devices=N_CORES)

    xs_d = nc.dram_tensor("xs", [B_PER, IN_DIM, L], BF16, kind="ExternalInput").ap()
    zp_d = nc.dram_tensor("zp", [B_PER, WIDTH, L], BF16, kind="ExternalInput").ap()
    wp_d = nc.dram_tensor("wp", [128, 1920], BF16, kind="ExternalInput").ap()
    w8_d = nc.dram_tensor("w8", [128, 2, 2, 128], FP8, kind="ExternalInput").ap()
    bp_d = nc.dram_tensor("bp", [128, 3], F32, kind="ExternalInput").ap()
    out_d = nc.dram_tensor("out", [B_PER, WIDTH, L], BF16,
                           kind="ExternalOutput").ap()

    # [b, (k p), l] -> [b, p, k, l]: a chunk slice is a [128, KC, ST] DMA
    # with contiguous rows
    xs_r = xs_d.rearrange("b (k p) l -> b p k l", p=128)

    with tile.TileContext(nc) as tc, ExitStack() as ctx:
        wpool = ctx.enter_context(tc.tile_pool(name="weights", bufs=1))
        io = ctx.enter_context(tc.tile_pool(name="io", bufs=5))
        obuf = ctx.enter_context(tc.tile_pool(name="obuf", bufs=2))
        acts = ctx.enter_context(tc.tile_pool(name="acts", bufs=2))
        ps_q = ctx.enter_context(tc.tile_pool(name="ps_q", bufs=2,
                                              space="PSUM"))
        ps_rz = ctx.enter_context(tc.tile_pool(name="ps_rz", bufs=1,
                                               space="PSUM"))

        # weights first (small, every matmul needs them), spread over the
        # three DMA-capable queues so the triggers don't serialize.
        w_sb = wpool.tile([128, 1920], BF16, tag="wp")
        nc.sync.dma_start(w_sb[:], wp_d[:])
        w8_sb = wpool.tile([128, 2, 2, 128], FP8, tag="w8")
        nc.gpsimd.dma_start(w8_sb[:], w8_d[:])
        b_sb = wpool.tile([128, 3], F32, tag="bp")
        nc.scalar.dma_start(b_sb[:], bp_d[:])

        # weight slices: per gate g (0=q,1=r,2=z): W chunks at
        # [:, g*512 + k*128 : +128], U at [:, 1536 + g*128 : +128]
        def wslice(g, k):
            return w_sb[:, g * 512 + k * 128: g * 512 + (k + 1) * 128]

        def uslice(g):
            return w_sb[:, 1536 + g * 128: 1536 + (g + 1) * 128]

        n_chunks = B_PER * (L // ST)
        tiles = {}

        def load_chunk(m):
            """Input DMAs for chunk m, plus the on-chip bf16->fp8 cast of
            xs on the (otherwise idle) GpSimd engine. The cast replaces an
            8.4MB HBM stream; it runs ~2 chunks ahead of use."""
            mb, ml = divmod(m, L // ST)
            ml *= ST
            xs_t = io.tile([128, KC, ST], BF16, tag="xs_t")
            nc.sync.dma_start(xs_t[:], xs_r[mb][:, :, ml:ml + ST])
            z_t = io.tile([128, ST], BF16, tag="z_t")
            nc.sync.dma_start(z_t[:], zp_d[mb][:, ml:ml + ST])
            x8_t = None
            if m > 0:  # chunk 0's r gate runs on the bf16 weights
                x8_t = io.tile([128, KC, ST], FP8, tag="x8_t")
            tiles[m] = (xs_t, x8_t, z_t)

        LOOKAHEAD = 2
        for m in range(min(LOOKAHEAD + 1, n_chunks)):
            load_chunk(m)

        for n in range(n_chunks):
            if n + LOOKAHEAD + 1 < n_chunks:
                load_chunk(n + LOOKAHEAD + 1)
            b_i, l0 = divmod(n, L // ST)
            l0 *= ST
            xs_t, x8_t, z_t = tiles.pop(n)
            ob = obuf.tile([128, ST], BF16, tag="ob")

            q_ps = ps_q.tile([128, ST], F32, tag="q_ps")
            r_ps = ps_rz.tile([128, ST], F32, tag="r_ps")
            zt_ps = ps_rz.tile([128, ST], F32, tag="zt_ps")

            def r_gate():
                # r gate: its sigmoid + r*z run on ACT/DVE while the q/zt
                # matmuls stream, so rz is ready well before the Uz matmuls
                # at the end of this supertile's stream. Chunk 0 uses the
                # bf16 Wr (pre-scaled x64 like the fp8 path) so the very
                # first matmul is gated by the xs DMA, not the later x8.
                if n == 0:
                    for k in range(KC):
                        w = wslice(1, k)
                        for h in range(2):
                            nc.tensor.matmul(
                                r_ps[:, h * NT:(h + 1) * NT], w,
                                xs_t[:, k, h * NT:(h + 1) * NT],
                                start=(k == 0), stop=False)
                else:
                    for k2 in range(2):
                        for h in range(2):
                            nc.tensor.matmul(
                                r_ps[:, h * NT:(h + 1) * NT], w8_sb[:, k2],
                                x8_t[:, 2 * k2:2 * k2 + 2,
                                     h * NT:(h + 1) * NT],
                                start=(k2 == 0), stop=False,
                                perf_mode=mybir.MatmulPerfMode.DoubleRow)
                ur = uslice(1)
                for h in range(2):
                    nc.tensor.matmul(r_ps[:, h * NT:(h + 1) * NT], ur,
                                     z_t[:, h * NT:(h + 1) * NT],
                                     start=False, stop=True)

            def q_gate():
                for k in range(KC):
                    w = wslice(0, k)
                    for h in range(2):
                        nc.tensor.matmul(
                            q_ps[:, h * NT:(h + 1) * NT], w,
                            xs_t[:, k, h * NT:(h + 1) * NT],
                            start=(k == 0), stop=False)
                uq = uslice(0)
                for h in range(2):
                    nc.tensor.matmul(q_ps[:, h * NT:(h + 1) * NT], uq,
                                     z_t[:, h * NT:(h + 1) * NT],
                                     start=False, stop=True)

            r_gate()
            q_gate()
            r_s = acts.tile([128, ST], BF16, tag="r_s")
            nc.scalar.activation(r_s[:], r_ps[:],
                                 mybir.ActivationFunctionType.Sigmoid,
                                 bias=b_sb[:, 1:2], scale=1.0 / WRS)
            rz = acts.tile([128, ST], BF16, tag="rz")
            nc.vector.tensor_mul(rz[:], r_s[:], z_t[:])
            # bf16->fp8 cast of FUTURE chunks' xs, in DVE's idle window
            # between rz (the only PE-gating DVE op) and the combine.
            # Chunk n casts chunk n+2 (n=0 also casts chunk 1), always >=2
            # chunks ahead of the fp8 r matmuls that consume it.
            cast_targets = (1, 2) if n == 0 else (n + 2,)
            for cm in cast_targets:
                if cm < n_chunks and tiles.get(cm, (None, None))[1] is not None:
                    nc.vector.tensor_scalar_add(tiles[cm][1][:],
                                                tiles[cm][0][:], 0.0)
            q_s = acts.tile([128, ST], BF16, tag="q_s")
            nc.scalar.activation(q_s[:], q_ps[:],
                                 mybir.ActivationFunctionType.Sigmoid,
                                 bias=b_sb[:, 0:1])

            # ---- zt gate: W part, then Uz@(r*z) at the stream tail
            for k in range(KC):
                w = wslice(2, k)
                for h in range(2):
                    nc.tensor.matmul(
                        zt_ps[:, h * NT:(h + 1) * NT], w,
                        xs_t[:, k, h * NT:(h + 1) * NT],
                        start=(k == 0), stop=False)
            uz = uslice(2)
            for h in range(2):
                nc.tensor.matmul(zt_ps[:, h * NT:(h + 1) * NT], uz,
                                 rz[:, h * NT:(h + 1) * NT],
                                 start=False, stop=True)

            # ---- epilogue: out = zt + q*(z - zt), bf16 on DVE. The last
            # chunk runs it in 512-halves so the serial tail chain
            # (tanh -> sub -> mul -> add -> store) pipelines.
            halves = ((0, ST),) if n < n_chunks - 1 else ((0, NT), (NT, NT))
            for h0, hw_ in halves:
                zt_s = acts.tile([128, hw_], BF16, tag=f"zt_s{h0}")
                nc.scalar.activation(zt_s[:], zt_ps[:, h0:h0 + hw_],
                                     mybir.ActivationFunctionType.Tanh,
                                     bias=b_sb[:, 2:3])
                diff = acts.tile([128, hw_], BF16, tag=f"diff{h0}")
                nc.vector.tensor_sub(diff[:], z_t[:, h0:h0 + hw_], zt_s[:])
                prod = acts.tile([128, hw_], BF16, tag=f"prod{h0}")
                nc.vector.tensor_mul(prod[:], q_s[:, h0:h0 + hw_], diff[:])
                nc.vector.tensor_add(ob[:, h0:h0 + hw_], zt_s[:], prod[:])
                nc.gpsimd.dma_start(out_d[b_i][:, l0 + h0:l0 + h0 + hw_],
                                    ob[:, h0:h0 + hw_])

    nc.compile()
    _module_cache[key] = nc
    return nc


def _pack_weights(inputs):
    # wp [128, 1920] bf16: per partition p:
    #   [g=q,r,z][k=0..3]: wp[p, g*512+k*128+o] = Wg_w[o, k*128+p]
    #   [g]: wp[p, 1536+g*128+o] = Ug_w[o, p]   (Ur pre-scaled by WRS)
    wp = np.empty((128, 1920), np.float32)
    bp = np.empty((128, 3), np.float32)
    for g, (wn, un, wbn, ubn) in enumerate((
        ("Wq_w", "Uq_w", "Wq_b", "Uq_b"),
        ("Wr_w", "Ur_w", "Wr_b", "Ur_b"),
        ("Wz_w", "Uz_w", "Wz_b", "Uz_b"),
    )):
        w = np.asarray(inputs[wn], np.float32)       # [128 out, 512 in]
        # Wr is pre-scaled like its fp8 twin (x64 is exact in bf16) --
        # chunk 0's r gate runs on this bf16 copy.
        ws = WRS if g == 1 else 1.0
        wp[:, g * 512:(g + 1) * 512] = ws * (
            w.reshape(128, KC, 128).transpose(2, 1, 0).reshape(128, 512))
        us = WRS if g == 1 else 1.0
        wp[:, 1536 + g * 128: 1536 + (g + 1) * 128] = (
            us * np.asarray(inputs[un], np.float32).T)
        bp[:, g] = (np.asarray(inputs[wbn], np.float32)
                    + np.asarray(inputs[ubn], np.float32))
    # w8 [128, k2, j, o] fp8: WRS * Wr_w[o, (2*k2+j)*128 + p]
    wr = np.asarray(inputs["Wr_w"], np.float32)      # [128, 512]
    w8 = (WRS * wr.reshape(128, 2, 2, 128).transpose(3, 1, 2, 0))
    return (np.ascontiguousarray(wp.astype(ml_dtypes.bfloat16)),
            np.ascontiguousarray(w8.astype(ml_dtypes.float8_e4m3)),
            np.ascontiguousarray(bp))


def _run(inputs, trace=False, **run_kwargs):
    xs = np.asarray(inputs["xs"], dtype=np.float32)
    zp = np.asarray(inputs["z_prev"], dtype=np.float32)
    assert xs.shape == (B, IN_DIM, L) and zp.shape == (B, WIDTH, L)
    xs_bf = np.ascontiguousarray(xs.astype(ml_dtypes.bfloat16))
    zp_bf = np.ascontiguousarray(zp.astype(ml_dtypes.bfloat16))
    wp, w8, bp = _pack_weights(inputs)

    nc = _build()
    in_maps = []
    for c in range(N_CORES):
        m = {"xs": np.ascontiguousarray(xs_bf[c * B_PER:(c + 1) * B_PER]),
             "zp": np.ascontiguousarray(zp_bf[c * B_PER:(c + 1) * B_PER]),
             "wp": wp, "w8": w8, "bp": bp}
        in_maps.append(m)

    res = run_bass_kernel_spmd(nc, in_maps, core_ids=list(range(N_CORES)),
                               trace=trace, **run_kwargs)
    out = np.concatenate(
        [np.asarray(res.results[c]["out"], dtype=np.float32)
         for c in range(N_CORES)], axis=0)
    return out, res


def kernel(**inputs):
    out, _ = _run(inputs, trace=False)
    return out


# revision 39
# speedup vs baseline: 9.7728x; 1.0138x over previous
"""GRUAggregation1d Trainium2 kernel.

Computes, for xs [B=16, 512, L=8192], z_prev [B, 128, L] (all fp32):
    q  = sigmoid(Wq@xs + Uq@z + bq)        (per position l, batch b)
    r  = sigmoid(Wr@xs + Ur@z + br)
    zt = tanh(Wz@xs + Uz@(r*z) + bz)
    out = q*z + (1-q)*zt

Sharding: data-parallel over batch. 8 cores x 2 batches each; weights
replicated.

Design (baseline ~160us -> ~123us):
- q/zt matmuls bf16 (fp8 measured out of tolerance on those paths); the
  r gate runs fp8 DoubleRow (K=256 per pass) with x64 pre-scaled weights,
  compensated via the sigmoid's scale; Ur is pre-scaled x64 in bf16 so
  its products land on the same PSUM scale. PSUM fp32, N=512 per matmul.
- Work unit: 1024-position supertile == DMA chunk (4KB xs rows, 2KB fp8
  rows). 5-deep input prefetch so the DMA stream never starves the PE
  after the initial fill. xs/z triggers on the sync queue, x8/out on
  gpsimd, scalar reserved for ACT (DMA triggers cost ~0.7us each).
- Per supertile the r gate is computed FIRST so r*z (DVE, bf16) is ready
  ~1.3us before the Uz matmuls at the stream tail -- no cross-supertile
  software pipeline needed, and the PE never waits on the r->r*z chain.
- Consecutive matmuls always target different PSUM banks; the two
  half-tile matmuls per weight are adjacent (stationary-weight reuse;
  LDWEIGHTS overlaps matmuls in HW).
- ACT ops are 1024 wide with fused bias (PSUM fp32 in, bf16 out); the
  combine runs in bf16 on DVE (2x rate); z_prev and out are bf16 in HBM
  (host casts), halving that traffic.
- All bf16 weights in one [128,1920] DMA; fp8 r-weights + biases in two
  tiny DMAs.
"""

from contextlib import ExitStack

import ml_dtypes
import numpy as np

import concourse.bass as bass
import concourse.mybir as mybir
import concourse.tile as tile
from concourse import bacc
from concourse.bass_utils import run_bass_kernel_spmd

B, IN_DIM, WIDTH, L = 16, 512, 128, 8192
N_CORES = 8
B_PER = B // N_CORES          # batches per core
KC = IN_DIM // 128            # K chunks for the W matmuls
NT = 512                      # positions per matmul (one PSUM bank)
ST = 1024                     # supertile / DMA chunk positions
F32 = mybir.dt.float32
BF16 = mybir.dt.bfloat16
FP8 = mybir.dt.float8e4
WRS = 64.0                    # r-gate fp8 weight pre-scale

_module_cache = {}


def _build():
    key = ("v11", ST)
    if key in _module_cache:
        return _module_cache[key]

    nc = bacc.Bacc("TRN2", target_bir_lowering=False, debug=False,
                   num_devices=N_CORES)

    xs_d = nc.dram_tensor("xs", [B_PER, IN_DIM, L], BF16, kind="ExternalInput").ap()
    zp_d = nc.dram_tensor("zp", [B_PER, WIDTH, L], BF16, kind="ExternalInput").ap()
    wp_d = nc.dram_tensor("wp", [128, 1920], BF16, kind="ExternalInput").ap()
    w8_d = nc.dram_tensor("w8", [128, 2, 2, 128], FP8, kind="ExternalInput").ap()
    bp_d = nc.dram_tensor("bp", [128, 3], F32, kind="ExternalInput").ap()
    out_d = nc.dram_tensor("out", [B_PER, WIDTH, L], BF16,
                           kind="ExternalOutput").ap()

    # [b, (k p), l] -> [b, p, k, l]: a chunk slice is a [128, KC, ST] DMA
    # with contiguous rows
    xs_r = xs_d.rearrange("b (k p) l -> b p k l", p=128)

    with tile.TileContext(nc) as tc, ExitStack() as ctx:
        wpool = ctx.enter_context(tc.tile_pool(name="weights", bufs=1))
        io = ctx.enter_context(tc.tile_pool(name="io", bufs=5))
        obuf = ctx.enter_context(tc.tile_pool(name="obuf", bufs=2))
        acts = ctx.enter_context(tc.tile_pool(name="acts", bufs=2))
        ps_q = ctx.enter_context(tc.tile_pool(name="ps_q", bufs=2,
                                              space="PSUM"))
        ps_rz = ctx.enter_context(tc.tile_pool(name="ps_rz", bufs=1,
                                               space="PSUM"))

        # weights first (small, every matmul needs them), spread over the
        # three DMA-capable queues so the triggers don't serialize.
        w_sb = wpool.tile([128, 1920], BF16, tag="wp")
        nc.sync.dma_start(w_sb[:], wp_d[:])
        w8_sb = wpool.tile([128, 2, 2, 128], FP8, tag="w8")
        nc.gpsimd.dma_start(w8_sb[:], w8_d[:])
        b_sb = wpool.tile([128, 3], F32, tag="bp")
        nc.scalar.dma_start(b_sb[:], bp_d[:])

        # weight slices: per gate g (0=q,1=r,2=z): W chunks at
        # [:, g*512 + k*128 : +128], U at [:, 1536 + g*128 : +128]
        def wslice(g, k):
            return w_sb[:, g * 512 + k * 128: g * 512 + (k + 1) * 128]

        def uslice(g):
            return w_sb[:, 1536 + g * 128: 1536 + (g + 1) * 128]

        n_chunks = B_PER * (L // ST)
        tiles = {}

        def load_chunk(m):
            """Input DMAs for chunk m, plus the on-chip bf16->fp8 cast of
            xs on the (otherwise idle) GpSimd engine. The cast replaces an
            8.4MB HBM stream; it runs ~2 chunks ahead of use."""
            mb, ml = divmod(m, L // ST)
            ml *= ST
            xs_t = io.tile([128, KC, ST], BF16, tag="xs_t")
            if m == 0:
                # two half DMAs: the PE starts on the first 512 positions
                # ~2.6us before the full tile would have landed
                nc.sync.dma_start(xs_t[:, :, 0:NT], xs_r[mb][:, :, ml:ml + NT])
                nc.sync.dma_start(xs_t[:, :, NT:ST],
                                  xs_r[mb][:, :, ml + NT:ml + ST])
            else:
                nc.sync.dma_start(xs_t[:], xs_r[mb][:, :, ml:ml + ST])
            z_t = io.tile([128, ST], BF16, tag="z_t")
            nc.sync.dma_start(z_t[:], zp_d[mb][:, ml:ml + ST])
            x8_t = None
            if m > 0:  # chunk 0's r gate runs on the bf16 weights
                x8_t = io.tile([128, KC, ST], FP8, tag="x8_t")
            tiles[m] = (xs_t, x8_t, z_t)

        LOOKAHEAD = 2
        for m in range(min(LOOKAHEAD + 1, n_chunks)):
            load_chunk(m)

        for n in range(n_chunks):
            if n + LOOKAHEAD + 1 < n_chunks:
                load_chunk(n + LOOKAHEAD + 1)
            b_i, l0 = divmod(n, L // ST)
            l0 *= ST
            xs_t, x8_t, z_t = tiles.pop(n)
            ob = obuf.tile([128, ST], BF16, tag="ob")

            q_ps = ps_q.tile([128, ST], F32, tag="q_ps")
            r_ps = ps_rz.tile([128, ST], F32, tag="r_ps")
            zt_ps = ps_rz.tile([128, ST], F32, tag="zt_ps")

            def r_gate():
                # r gate: its sigmoid + r*z run on ACT/DVE while the q/zt
                # matmuls stream, so rz is ready well before the Uz matmuls
                # at the end of this supertile's stream. Chunk 0 uses the
                # bf16 Wr (pre-scaled x64 like the fp8 path) so the very
                # first matmul is gated by the xs DMA, not the later x8.
                if n == 0:
                    # h-major so all first-half matmuls precede any
                    # second-half ones (the halves arrive as two DMAs)
                    for h in range(2):
                        for k in range(KC):
                            nc.tensor.matmul(
                                r_ps[:, h * NT:(h + 1) * NT], wslice(1, k),
                                xs_t[:, k, h * NT:(h + 1) * NT],
                                start=(k == 0), stop=False)
                else:
                    for k2 in range(2):
                        for h in range(2):
                            nc.tensor.matmul(
                                r_ps[:, h * NT:(h + 1) * NT], w8_sb[:, k2],
                                x8_t[:, 2 * k2:2 * k2 + 2,
                                     h * NT:(h + 1) * NT],
                                start=(k2 == 0), stop=False,
                                perf_mode=mybir.MatmulPerfMode.DoubleRow)
                ur = uslice(1)
                for h in range(2):
                    nc.tensor.matmul(r_ps[:, h * NT:(h + 1) * NT], ur,
                                     z_t[:, h * NT:(h + 1) * NT],
                                     start=False, stop=True)

            def q_gate():
                hk = ([(h, k) for h in range(2) for k in range(KC)] if n == 0
                      else [(h, k) for k in range(KC) for h in range(2)])
                for h, k in hk:
                    nc.tensor.matmul(
                        q_ps[:, h * NT:(h + 1) * NT], wslice(0, k),
                        xs_t[:, k, h * NT:(h + 1) * NT],
                        start=(k == 0), stop=False)
                uq = uslice(0)
                for h in range(2):
                    nc.tensor.matmul(q_ps[:, h * NT:(h + 1) * NT], uq,
                                     z_t[:, h * NT:(h + 1) * NT],
                                     start=False, stop=True)

            r_gate()
            q_gate()
            r_s = acts.tile([128, ST], BF16, tag="r_s")
            nc.scalar.activation(r_s[:], r_ps[:],
                                 mybir.ActivationFunctionType.Sigmoid,
                                 bias=b_sb[:, 1:2], scale=1.0 / WRS)
            rz = acts.tile([128, ST], BF16, tag="rz")
            nc.vector.tensor_mul(rz[:], r_s[:], z_t[:])
            # bf16->fp8 cast of FUTURE chunks' xs, in DVE's idle window
            # between rz (the only PE-gating DVE op) and the combine.
            # Chunk n casts chunk n+2 (n=0 also casts chunk 1), always >=2
            # chunks ahead of the fp8 r matmuls that consume it.
            cast_targets = (1, 2) if n == 0 else (n + 2,)
            for cm in cast_targets:
                if cm < n_chunks and tiles.get(cm, (None, None))[1] is not None:
                    nc.vector.tensor_scalar_add(tiles[cm][1][:],
                                                tiles[cm][0][:], 0.0)
            q_s = acts.tile([128, ST], BF16, tag="q_s")
            nc.scalar.activation(q_s[:], q_ps[:],
                                 mybir.ActivationFunctionType.Sigmoid,
                                 bias=b_sb[:, 0:1])

            # ---- zt gate: W part, then Uz@(r*z) at the stream tail
            for k in range(KC):
                w = wslice(2, k)
                for h in range(2):
                    nc.tensor.matmul(
                        zt_ps[:, h * NT:(h + 1) * NT], w,
                        xs_t[:, k, h * NT:(h + 1) * NT],
                        start=(k == 0), stop=False)
            uz = uslice(2)
            for h in range(2):
                nc.tensor.matmul(zt_ps[:, h * NT:(h + 1) * NT], uz,
                                 rz[:, h * NT:(h + 1) * NT],
                                 start=False, stop=True)

            # ---- epilogue: out = zt + q*(z - zt), bf16 on DVE. The last
            # chunk runs it in 512-halves so the serial tail chain
            # (tanh -> sub -> mul -> add -> store) pipelines.
            halves = ((0, ST),) if n < n_chunks - 1 else ((0, NT), (NT, NT))
            for h0, hw_ in halves:
                zt_s = acts.tile([128, hw_], BF16, tag=f"zt_s{h0}")
                nc.scalar.activation(zt_s[:], zt_ps[:, h0:h0 + hw_],
                                     mybir.ActivationFunctionType.Tanh,
                                     bias=b_sb[:, 2:3])
                diff = acts.tile([128, hw_], BF16, tag=f"diff{h0}")
                nc.vector.tensor_sub(diff[:], z_t[:, h0:h0 + hw_], zt_s[:])
                prod = acts.tile([128, hw_], BF16, tag=f"prod{h0}")
                nc.vector.tensor_mul(prod[:], q_s[:, h0:h0 + hw_], diff[:])
                nc.vector.tensor_add(ob[:, h0:h0 + hw_], zt_s[:], prod[:])
                nc.gpsimd.dma_start(out_d[b_i][:, l0 + h0:l0 + h0 + hw_],
                                    ob[:, h0:h0 + hw_])

    nc.compile()
    _module_cache[key] = nc
    return nc


def _pack_weights(inputs):
    # wp [128, 1920] bf16: per partition p:
    #   [g=q,r,z][k=0..3]: wp[p, g*512+k*128+o] = Wg_w[o, k*128+p]
    #   [g]: wp[p, 1536+g*128+o] = Ug_w[o, p]   (Ur pre-scaled by WRS)
    wp = np.empty((128, 1920), np.float32)
    bp = np.empty((128, 3), np.float32)
    for g, (wn, un, wbn, ubn) in enumerate((
        ("Wq_w", "Uq_w", "Wq_b", "Uq_b"),
        ("Wr_w", "Ur_w", "Wr_b", "Ur_b"),
        ("Wz_w", "Uz_w", "Wz_b", "Uz_b"),
    )):
        w = np.asarray(inputs[wn], np.float32)       # [128 out, 512 in]
        # Wr is pre-scaled like its fp8 twin (x64 is exact in bf16) --
        # chunk 0's r gate runs on this bf16 copy.
        ws = WRS if g == 1 else 1.0
        wp[:, g * 512:(g + 1) * 512] = ws * (
            w.reshape(128, KC, 128).transpose(2, 1, 0).reshape(128, 512))
        us = WRS if g == 1 else 1.0
        wp[:, 1536 + g * 128: 1536 + (g + 1) * 128] = (
            us * np.asarray(inputs[un], np.float32).T)
        bp[:, g] = (np.asarray(inputs[wbn], np.float32)
                    + np.asarray(inputs[ubn], np.float32))
    # w8 [128, k2, j, o] fp8: WRS * Wr_w[o, (2*k2+j)*128 + p]
    wr = np.asarray(inputs["Wr_w"], np.float32)      # [128, 512]
    w8 = (WRS * wr.reshape(128, 2, 2, 128).transpose(3, 1, 2, 0))
    return (np.ascontiguousarray(wp.astype(ml_dtypes.bfloat16)),
            np.ascontiguousarray(w8.astype(ml_dtypes.float8_e4m3)),
            np.ascontiguousarray(bp))


def _run(inputs, trace=False, **run_kwargs):
    xs = np.asarray(inputs["xs"], dtype=np.float32)
    zp = np.asarray(inputs["z_prev"], dtype=np.float32)
    assert xs.shape == (B, IN_DIM, L) and zp.shape == (B, WIDTH, L)
    xs_bf = np.ascontiguousarray(xs.astype(ml_dtypes.bfloat16))
    zp_bf = np.ascontiguousarray(zp.astype(ml_dtypes.bfloat16))
    wp, w8, bp = _pack_weights(inputs)

    nc = _build()
    in_maps = []
    for c in range(N_CORES):
        m = {"xs": np.ascontiguousarray(xs_bf[c * B_PER:(c + 1) * B_PER]),
             "zp": np.ascontiguousarray(zp_bf[c * B_PER:(c + 1) * B_PER]),
             "wp": wp, "w8": w8, "bp": bp}
        in_maps.append(m)

    res = run_bass_kernel_spmd(nc, in_maps, core_ids=list(range(N_CORES)),
                               trace=trace, **run_kwargs)
    out = np.concatenate(
        [np.asarray(res.results[c]["out"], dtype=np.float32)
         for c in range(N_CORES)], axis=0)
    return out, res


def kernel(**inputs):
    out, _ = _run(inputs, trace=False)
    return out
